# revision 2
# baseline (speedup 1.0000x reference)
"""Two-phase Trainium2 Bass kernel for nn_DiffusionStar (retrieval_knn).

eps_star = (x - sqrt(ab) * weighted_x) / sqrt(1 - ab), weighted_x the
softmax-weighted train-set average under the Gaussian kernel.

vs the baseline single-pass online-softmax kernel, this version:
  - Phase 1 streams ONLY the transposed fp16 copy (a_hi, 78.6MB/core),
    computing all logits into an SBUF-resident buffer [B, 12800] and the
    row max. No fp8 residual stream (a_lo dropped: logit err sigma ~0.05,
    corrected on host by exact top-K rescoring).
  - Phase 2 streams the natural-layout copy in fp8 (b8, 39.3MB/core;
    half the baseline's fp16 bytes), exponentiates the stored logits
    against the FINAL max (no online rescaling), and accumulates the
    weighted sum directly in PSUM across all tiles.
  - Host merges per-core (W, m, s) and applies an exact top-K correction
    using the exported logits: subtract the device's reconstructed fp8/fp16
    contributions for the top-K rows per query, add exact f64 ones.

Per-core HBM traffic drops 197MB -> 118MB and PE streams drop ~38k ->
~26k cycles/tile.
"""

import contextlib
import os

import ml_dtypes
import numpy as np

from concourse import bacc, bass, mybir, tile
from concourse import bass_utils

FP16 = mybir.dt.float16
FP8 = mybir.dt.float8e4
F32 = mybir.dt.float32
NP_FP8 = ml_dtypes.float8_e4m3

B = 32          # queries
D = 3072        # feature dim (c*h*w)
N = 100000      # train points
N_CORES = 8
N_SHARD = N // N_CORES          # 12500
TILE = 512
N_TILES = (N_SHARD + TILE - 1) // TILE   # 25
N_PAD = N_TILES * TILE                   # 12800
KC = D // 128                            # 24 contraction chunks
PAD_BIAS = -30000.0                      # logit bias for padded rows
TOP_K = 32                               # host-side exact rescore depth

P_FP8 = True       # cast softmax weights to fp8 for the W matmul
DOUBLE_ROW = True  # fp8 DoubleRow perf mode on the W matmul
A_FP8 = os.environ.get("K2_A_FP8", "1") == "1"
                   # pure-fp8 a-side (39.3MB vs 78.6MB); logit noise sigma~8
                   # is confined to the negligible softmax tail by the exact
                   # top-K host rescore (sim: rel err 1.4e-13)
LO_SCALE = 64.0    # scale of the x-side fp8 lo-residual row
B8_Q = os.environ.get("K2_B8_Q", "gs")  # b8 queue plan: g|s|y|gs|sy ...


def build_nc(n_tiles=N_TILES, repeat=1, skip_compute=False, skip_dma=False,
             p_fp8=P_FP8, double_row=DOUBLE_ROW, a_fp8=A_FP8,
             phase1_only=False, phase2_only=False):
    nc = bacc.Bacc("TRN2", target_bir_lowering=False, debug=False, num_devices=1)

    a_dt = FP8 if a_fp8 else FP16
    # transposed copy, partition-major per tile: [tile, p, k, n]
    a_hi = nc.dram_tensor(
        "a_hi", [n_tiles, 128, KC, TILE], a_dt, kind="ExternalInput"
    ).ap()
    a_b = nc.dram_tensor("a_b", [2, n_tiles, TILE], FP16, kind="ExternalInput").ap()
    # natural-layout fp8 copy: [tile, p, cb, d]
    b8 = nc.dram_tensor("b8", [n_tiles, 128, 4, D], FP8, kind="ExternalInput").ap()
    xw16 = nc.dram_tensor("xw16", [KC, 128, 64], a_dt, kind="ExternalInput").ap()
    xwb = nc.dram_tensor("xwb", [2, 64], FP16, kind="ExternalInput").ap()
    ident = nc.dram_tensor("ident", [32, 32], F32, kind="ExternalInput").ap()

    w_out = nc.dram_tensor("w_out", [B, D], F32, kind="ExternalOutput").ap()
    m_out = nc.dram_tensor("m_out", [B, 1], F32, kind="ExternalOutput").ap()
    s_out = nc.dram_tensor("s_out", [B, 1], F32, kind="ExternalOutput").ap()
    l_out = nc.dram_tensor(
        "l_out", [B, n_tiles * TILE], F32, kind="ExternalOutput"
    ).ap()

    p_dt = FP8 if p_fp8 else FP16
    use_dr = double_row and p_fp8

    with tile.TileContext(nc) as tc, contextlib.ExitStack() as st:
        const = st.enter_context(tc.tile_pool(name="const", bufs=1))
        apool = st.enter_context(tc.tile_pool(name="apool", bufs=3))
        bpool = st.enter_context(tc.tile_pool(name="bpool", bufs=4))
        small = st.enter_context(tc.tile_pool(name="small", bufs=4))
        pwork = st.enter_context(tc.tile_pool(name="pwork", bufs=3))
        # one shared double-buffered scratch bank pool: holds `cross` during
        # phase 1 and the p-transpose during phase 2 (PSUM: 2 + 6 = 8 banks)
        ps_scr = st.enter_context(tc.tile_pool(name="ps_scr", bufs=2, space="PSUM"))
        ps_w = st.enter_context(tc.tile_pool(name="ps_w", bufs=1, space="PSUM"))

        xw16_s = const.tile([128, KC, 64], a_dt)
        nc.sync.dma_start(xw16_s[:], xw16.rearrange("k p j -> p k j"))
        xwb_s = const.tile([2, 64], FP16)
        nc.sync.dma_start(xwb_s[:], xwb)
        ident_s = const.tile([32, 32], F32)
        nc.sync.dma_start(ident_s[:], ident)
        c64 = const.tile([B, 1], F32)
        nc.vector.memset(c64[:], 1.0 / LO_SCALE)

        # full bias rows resident: one DMA instead of 25 small ones
        ab_s = const.tile([2, n_tiles * TILE], FP16)
        nc.sync.dma_start(ab_s[:], a_b.rearrange("r t n -> r (t n)"))

        logbuf = const.tile([B, n_tiles * TILE], F32)
        mts = const.tile([B, n_tiles], F32)
        rsbuf = const.tile([B, n_tiles], F32)
        m_run = const.tile([B, 1], F32)
        s_run = const.tile([B, 1], F32)
        wsb = const.tile([B, D], F32)

        for r in range(repeat):
            nc.vector.memset(m_run[:], -1e30)
            nc.vector.memset(s_run[:], 0.0)

            # ---------------- phase 1: logits ----------------
            for i in range(n_tiles) if not phase2_only else []:
                a16_t = apool.tile([128, KC, TILE], a_dt, tag="a16")
                if not skip_dma:
                    # alternate queues so the a-stream isn't capped by one
                    # DGE ring's throughput
                    eng = nc.sync if i % 2 == 0 else nc.scalar
                    eng.dma_start(a16_t[:], a_hi[i])
                else:
                    nc.vector.memset(a16_t[:, 0, 0:2], 0.0)
                if skip_compute:
                    dmy = small.tile([128, 1], F32, tag="dmy")
                    nc.vector.reduce_max(
                        dmy[:], a16_t[:, 0, 0:8], axis=mybir.AxisListType.X
                    )
                    continue

                scr = ps_scr.tile([128, TILE], F32, tag="scr")
                if a_fp8:
                    for k in range(0, KC, 2):
                        nc.tensor.matmul(
                            scr[0:64, :], xw16_s[:, k : k + 2, :],
                            a16_t[:, k : k + 2, :],
                            start=(k == 0), stop=False,
                            perf_mode=mybir.MatmulPerfMode.DoubleRow,
                        )
                else:
                    for k in range(KC):
                        nc.tensor.matmul(
                            scr[0:64, :], xw16_s[:, k], a16_t[:, k],
                            start=(k == 0), stop=False,
                        )
                nc.tensor.matmul(
                    scr[0:64, :], xwb_s[:], ab_s[:, i * TILE : (i + 1) * TILE],
                    start=False, stop=True,
                )

                lslice = logbuf[:, i * TILE : (i + 1) * TILE]
                crossb = pwork.tile([B, TILE], F32, tag="crossb")
                nc.scalar.copy(crossb[:], scr[B:64, :])
                if a_fp8:
                    # logits = hi + lo/LO_SCALE (lo rows carry the x-side
                    # fp8 residual product)
                    nc.vector.scalar_tensor_tensor(
                        lslice, crossb[:], c64[:], scr[0:B, :],
                        mybir.AluOpType.mult, mybir.AluOpType.add,
                    )
                else:
                    nc.vector.tensor_add(lslice, scr[0:B, :], crossb[:])

                # per-tile max into a buffer; one final reduce (no serial chain)
                nc.vector.reduce_max(
                    mts[:, i : i + 1], lslice, axis=mybir.AxisListType.X
                )

            if skip_compute:
                # still stream b8 for the DMA-only bench
                for i in range(n_tiles):
                    b8_t = bpool.tile([128, 4, D], FP8, tag="b8")
                    if not skip_dma:
                        qmap = {"g": nc.gpsimd, "s": nc.scalar, "y": nc.sync}
                        qeng = qmap[B8_Q[i % len(B8_Q)]]
                        qeng.dma_start(b8_t[:], b8[i])
                    else:
                        nc.vector.memset(b8_t[:, 0, 0:2], 0.0)
                    dmy = small.tile([128, 1], F32, tag="dmy")
                    nc.vector.reduce_max(
                        dmy[:], b8_t[:, 0, 0:8], axis=mybir.AxisListType.X
                    )
                continue

            if phase2_only and r == 0:
                nc.vector.memset(logbuf[:], 0.0)
                nc.vector.memset(mts[:], 0.0)
            nc.vector.reduce_max(m_run[:], mts[:], axis=mybir.AxisListType.X)
            negm = small.tile([B, 1], F32, tag="negm")
            nc.vector.tensor_scalar_mul(negm[:], m_run[:], -1.0)

            # ---------------- phase 2: exp + weighted sum ----------------
            wp = ps_w.tile([B, D], F32, tag="wp")
            for i in range(n_tiles) if not phase1_only else []:
                b8_t = bpool.tile([128, 4, D], FP8, tag="b8")
                if not skip_dma:
                    qmap = {"g": nc.gpsimd, "s": nc.scalar, "y": nc.sync}
                    qeng = qmap[B8_Q[i % len(B8_Q)]]
                    qeng.dma_start(b8_t[:], b8[i])
                else:
                    nc.vector.memset(b8_t[:, 0, 0:2], 0.0)

                p = pwork.tile([B, TILE], F32, tag="p")
                nc.scalar.activation(
                    p[:], logbuf[:, i * TILE : (i + 1) * TILE],
                    mybir.ActivationFunctionType.Exp,
                    bias=negm[:], scale=1.0, accum_out=rsbuf[:, i : i + 1],
                )

                pTp = ps_scr.tile([128, TILE], F32, tag="scr")
                for cb in range(4):
                    nc.tensor.transpose(
                        pTp[:, cb * 32 : (cb + 1) * 32],
                        p[:, cb * 128 : (cb + 1) * 128],
                        ident_s[:],
                    )
                # [128, 4, 32]: dim1 = n-subchunk index, kept explicit so the
                # DoubleRow lhsT AP carries the required [2, 32] structure
                pT = pwork.tile([128, 4, 32], p_dt, tag="pTq")
                nc.vector.tensor_copy(pT[:], pTp[:, 0:128])

                if use_dr:
                    for cb in range(0, 4, 2):
                        for jb in range(D // 512):
                            sl = slice(jb * 512, (jb + 1) * 512)
                            nc.tensor.matmul(
                                wp[:, sl],
                                pT[:, cb : cb + 2, :],
                                b8_t[:, cb : cb + 2, sl],
                                start=(i == 0 and cb == 0),
                                stop=(i == n_tiles - 1 and cb == 2),
                                perf_mode=mybir.MatmulPerfMode.DoubleRow,
                            )
                else:
                    for cb in range(4):
                        for jb in range(D // 512):
                            sl = slice(jb * 512, (jb + 1) * 512)
                            nc.tensor.matmul(
                                wp[:, sl],
                                pT[:, cb, :],
                                b8_t[:, cb, sl],
                                start=(i == 0 and cb == 0),
                                stop=(i == n_tiles - 1 and cb == 3),
                            )

            if not phase1_only:
                nc.vector.reduce_sum(s_run[:], rsbuf[:], axis=mybir.AxisListType.X)
                nc.vector.tensor_copy(wsb[:], wp[:])

        if not skip_compute:
            if not phase1_only:
                nc.sync.dma_start(w_out, wsb[:])
                nc.sync.dma_start(s_out, s_run[:])
            nc.sync.dma_start(m_out, m_run[:])
            nc.sync.dma_start(l_out, logbuf[:])
        else:
            nc.sync.dma_start(m_out, m_run[:])

    nc.compile()
    return nc


_NC_CACHE = {}


def _get_nc():
    if "main" not in _NC_CACHE:
        _NC_CACHE["main"] = build_nc()
    return _NC_CACHE["main"]


LAST_RESULT = None
LAST_IN_MAPS = None


def _prep_in_maps(x, train_data, alphas_cumprod, t_idx):
    ab = float(alphas_cumprod[t_idx])
    s_ab = np.sqrt(ab)
    one_minus = 1.0 - ab
    coefA = s_ab / one_minus
    coefB = ab / (2.0 * one_minus)

    xf = x.reshape(B, D).astype(np.float64)
    xs = coefA * xf

    if A_FP8:
        xs_hi = xs.astype(NP_FP8)
        xs_lo = ((xs - xs_hi.astype(np.float64)) * LO_SCALE).astype(NP_FP8)
        xw16 = np.zeros((KC, 128, 64), NP_FP8)
    else:
        xs_hi = xs.astype(np.float16)
        xs_lo = (xs - xs_hi.astype(np.float64)).astype(np.float16)
        xw16 = np.zeros((KC, 128, 64), np.float16)
    for k in range(KC):
        sl = slice(k * 128, (k + 1) * 128)
        xw16[k, :, 0:B] = xs_hi[:, sl].T
        xw16[k, :, B:64] = xs_lo[:, sl].T
    xwb = np.zeros((2, 64), np.float16)
    xwb[0, 0:B] = 1.0
    xwb[1, 0:B] = 1.0
    ident = np.eye(32, dtype=np.float32)

    tf = train_data.reshape(N, D)
    in_maps = []
    t_sq_all = np.empty(N, np.float64)
    for c in range(N_CORES):
        shard = tf[c * N_SHARD : (c + 1) * N_SHARD].astype(np.float32)
        t_pad = np.zeros((N_PAD, D), np.float32)
        t_pad[:N_SHARD] = shard

        td = t_pad.astype(np.float64)
        t_sq = np.einsum("nd,nd->n", td, td)
        t_sq_all[c * N_SHARD : (c + 1) * N_SHARD] = t_sq[:N_SHARD]
        bias = -coefB * (t_sq - float(D))
        bias[N_SHARD:] = PAD_BIAS

        At = td.T                                    # [D, N_PAD]
        A_hi16 = At.astype(NP_FP8 if A_FP8 else np.float16)
        a_hi = np.ascontiguousarray(
            A_hi16.reshape(KC, 128, N_TILES, TILE).transpose(2, 1, 0, 3)
        )

        bias_hi = bias.astype(np.float16)
        bias_lo = (bias - bias_hi.astype(np.float64)).astype(np.float16)
        a_b = (
            np.stack([bias_hi, bias_lo])
            .reshape(2, N_TILES, TILE)
            .astype(np.float16)
        )

        b8 = np.ascontiguousarray(
            t_pad.astype(NP_FP8).reshape(N_TILES, 4, 128, D).transpose(0, 2, 1, 3)
        )

        in_maps.append(
            dict(a_hi=a_hi, a_b=a_b, b8=b8, xw16=xw16, xwb=xwb, ident=ident)
        )
    return in_maps, t_sq_all, (ab, s_ab, one_minus, coefA, coefB)


def kernel(x, train_data, alphas_cumprod, t):
    x = np.asarray(x)
    train_data = np.asarray(train_data)
    alphas_cumprod = np.asarray(alphas_cumprod)
    t_idx = int(np.asarray(t))

    in_maps, t_sq_all, (ab, s_ab, one_minus, coefA, coefB) = _prep_in_maps(
        x, train_data, alphas_cumprod, t_idx
    )
    inv = 1.0 / np.sqrt(one_minus)
    xf = x.reshape(B, D).astype(np.float64)
    tf = train_data.reshape(N, D)

    nc = _get_nc()
    res = bass_utils.run_bass_kernel_spmd(nc, in_maps, core_ids=list(range(N_CORES)))
    global LAST_RESULT, LAST_IN_MAPS
    LAST_RESULT = res
    LAST_IN_MAPS = in_maps

    Wc = np.stack([r["w_out"] for r in res.results]).astype(np.float64)  # [8,B,D]
    mc = np.stack([r["m_out"][:, 0] for r in res.results]).astype(np.float64)
    sc = np.stack([r["s_out"][:, 0] for r in res.results]).astype(np.float64)
    lc = np.stack([r["l_out"] for r in res.results])                     # [8,B,NP] f32

    M = mc.max(0)                                    # [B]
    fac = np.exp(mc - M[None, :])                    # [8, B]
    W_tot = np.einsum("cb,cbd->bd", fac, Wc)
    s_tot = (fac * sc).sum(0)                        # [B]

    # ---- exact top-K rescore from exported logits ----
    # global row n of core c local j (j < N_SHARD) maps to train row
    # c*N_SHARD + j; padded rows carry PAD_BIAS and are never selected.
    lflat = lc[:, :, :N_SHARD].transpose(1, 0, 2).reshape(B, N)  # [B, N] f32
    np_pdt = NP_FP8 if P_FP8 else np.float16
    for b in range(B):
        idx = np.argpartition(lflat[b], -TOP_K)[-TOP_K:]
        c_arr = idx // N_SHARD
        rows = tf[idx].astype(np.float64)
        rows8 = tf[idx].astype(np.float32).astype(NP_FP8).astype(np.float64)
        l_dev = lflat[b, idx].astype(np.float64)
        # device-reconstructed contribution
        p32 = np.exp(
            (lflat[b, idx].astype(np.float32) - mc[c_arr, b].astype(np.float32))
            .astype(np.float32)
            .astype(np.float64)
        ).astype(np.float32)
        pq = p32.astype(np_pdt).astype(np.float64)
        f = fac[c_arr, b]
        W_tot[b] -= (f * pq) @ rows8
        s_tot[b] -= float((f * p32.astype(np.float64)).sum())
        # exact contribution
        l_ex = coefA * (rows @ xf[b]) - coefB * (t_sq_all[idx] - float(D))
        p_ex = np.exp(l_ex - M[b])
        W_tot[b] += p_ex @ rows
        s_tot[b] += float(p_ex.sum())

    weighted = W_tot / s_tot[:, None]                # [B, D]
    out = inv * xf - (s_ab * inv) * weighted
    return out.reshape(x.shape).astype(np.float32)
